# revision 1
# baseline (speedup 1.0000x reference)
"""Dot-product stereo cost volume on 8 Trainium2 NeuronCores.

cost[b, d, y, x] = sum_c left[b,c,y,x] * right[b,c,y,x-d], zeros where x-d < 0.
Shapes: left/right [4, 128, 192, 640] fp32, D = 96 -> out [4, 96, 192, 640] fp32.

Strategy
--------
Sharding: 8 cores <- (b, y-half): core k handles batch k//2, rows 96*(k%2)..+96.
No halo needed (disparity shifts are along W only).

Per (y) row the math is a banded Gram matrix: G_y[x', x] = sum_c R[c,x'] L[c,x],
and cost[d, y, x] = G_y[x-d, x].  The PE computes G in M=64-row tiles:
tile t covers x' in [64t, 64t+64), x in [64t, 64t+160) (since d <= 95, every
needed (x', x) pair with x' in that 64-block satisfies 0 <= x - x' <= 159).
Two such tiles stack into one [128, 160] PSUM tile via tile_position column
groups.  The raw rect tiles stream to a DRAM scratch buffer; the diagonal
reindex (d = x - x') is absorbed into the host-side unshard with one
precomputed fancy index (a diagonal of G is not expressible as a DMA access
pattern: SBUF-side APs cannot couple partition and byte offsets, and burst
contiguity runs along d on the source but along x in the output layout).

M=64 balances the two rooflines: fp32 matmul streams at 4 cycles/row
regardless of M (XBUS-bandwidth-bound, so column-group concurrency does not
help fp32), giving PE time ~ (W/M)(M+96)*4cyc/row, while scratch-write bytes
grow as W*(M+96).  M=64 puts PE at ~261us and DMA at ~290us per core.
Scratch is y-pair-major so each store is one plain ~800KB contiguous DMA
(input loads are 640KB, 2 rows each, on the other HWDGE ring); measured
~270-320us per invocation against a ~280us combined HBM roofline.
"""

import sys

if "/opt/trn_rl_repo" not in sys.path:
    sys.path.insert(0, "/opt/trn_rl_repo")

import numpy as np

B, C, H, W = 4, 128, 192, 640
D = 96
HSH = H // 2          # rows per core
MT = 64               # M (x') tile height
NW = MT + 96          # free (x) tile width per matmul
NT = W // MT          # x'-tiles per row
ST = 128 // MT        # tiles stacked per psum tile (col groups)
NPS = NT // ST        # psum tiles per row

_compiled = None


def _build(repeat=1, do_load=True, do_pe=True, do_store=True,
           yb=2, lbufs=4, sbufs=4, pbufs=6):
    import contextlib
    import concourse.bacc as bacc
    import concourse.tile as tile
    import concourse.mybir as mybir

    nc = bacc.Bacc("TRN2", target_bir_lowering=False, debug=False, num_devices=8)
    f32 = mybir.dt.float32

    left_ap = nc.dram_tensor("left", [C, HSH, W], f32, kind="ExternalInput").ap()
    right_ap = nc.dram_tensor("right", [C, HSH, W], f32, kind="ExternalInput").ap()
    # y-pair-major, p-major layout: one store covers 2 rows as a single
    # plain [128, 2*NPS*NW] contiguous-per-partition DMA
    scr_ap = nc.dram_tensor(
        "scr", [HSH // 2, 128, 2 * NPS * NW], f32, kind="ExternalOutput"
    ).ap()

    WPAD = W + 96  # L is zero-padded on the right so every rhs window is full

    YB = yb  # y rows loaded per input DMA

    with tile.TileContext(nc) as tc:
        with (
            tc.tile_pool(name="lpool", bufs=lbufs) as lpool,
            tc.tile_pool(name="rpool", bufs=lbufs) as rpool,
            tc.tile_pool(name="stage", bufs=sbufs) as stage_pool,
            tc.tile_pool(name="psum", bufs=pbufs, space="PSUM") as psum_pool,
        ):
            rep_ctx = (
                tc.For_i(0, repeat, 1) if repeat > 1 else contextlib.nullcontext()
            )
            with rep_ctx:
                for y0 in range(0, HSH, YB):
                    # [c, (y pair, x)] input tiles; loads on the ACT HWDGE
                    # ring so they round-robin against stores on the SP ring
                    lt = lpool.tile([128, YB * WPAD], f32, name=f"lt_{y0}", tag="lt")
                    rt = rpool.tile([128, YB * W], f32, name=f"rt_{y0}", tag="rt")
                    lt3 = lt.rearrange("c (y w) -> c y w", y=YB)
                    if not do_load and do_pe:
                        # ablation only: make Tile see a write so reads schedule
                        nc.vector.memset(lt[:, 0:8], 0.0)
                        nc.vector.memset(rt[:, 0:8], 0.0)
                    if do_load:
                        nc.scalar.dma_start(lt3[:, :, 0:W], left_ap[:, y0 : y0 + YB, :])
                        nc.vector.memset(lt3[:, :, W:WPAD], 0.0)
                        nc.scalar.dma_start(
                            rt.rearrange("c (y w) -> c y w", y=YB),
                            right_ap[:, y0 : y0 + YB, :],
                        )

                    RW = NPS * NW  # per-row stage width
                    st = stage_pool.tile([128, 2 * RW], f32, name=f"st_{y0}", tag="st")
                    if not do_pe and do_store:
                        nc.vector.memset(st[:, 0:8], 0.0)
                    for yi in range(YB):
                        y = y0 + yi
                        if do_pe:
                            for s in range(NPS):
                                ps = psum_pool.tile([128, NW], f32, name=f"ps_{y}_{s}", tag="ps")
                                for u in range(ST):
                                    t = ST * s + u
                                    q0 = yi * WPAD + MT * t
                                    nc.tensor.matmul(
                                        ps[MT * u : MT * (u + 1), :],
                                        lhsT=rt[:, yi * W + MT * t : yi * W + MT * t + MT],
                                        rhs=lt[:, q0 : q0 + NW],
                                        start=True,
                                        stop=True,
                                        tile_position=(0, MT * u),
                                    )
                                nc.vector.tensor_copy(
                                    st[:, yi * RW + s * NW : yi * RW + (s + 1) * NW],
                                    ps[:],
                                )
                    if do_store:
                        # one plain contiguous store per y-pair; the right-edge
                        # junk of the last psum tile block rides along (host
                        # never reads it) -- keeping the AP trivial
                        nc.sync.dma_start(scr_ap[y0 // 2], st[:])

    nc.compile()
    return nc


def _host_index():
    """idx[d, x] -> flat offset into scr[y] (= [128*NPS*NW]) holding G[x-d, x].

    Valid only where x >= d; mask handles the rest.
    """
    d = np.arange(D)[:, None]
    x = np.arange(W)[None, :]
    xp = x - d                       # x' = x - d
    t = np.maximum(xp, 0) // MT      # x'-tile
    q = np.maximum(xp, 0) - MT * t   # row within tile
    s = t // ST                      # psum tile
    u = t - ST * s                   # col group within psum tile
    f = x - MT * t                   # col within tile (< NW always)
    p = MT * u + q                   # psum partition
    idx = (p * NPS + s) * NW + f     # scr[y] is [128 p, NPS s, NW f]
    mask = (x >= d)
    return idx.astype(np.int64), mask


def kernel(left, right, num_disparities):
    global _compiled
    left = np.asarray(left)
    right = np.asarray(right)
    assert int(num_disparities) == D
    assert left.shape == (B, C, H, W) and right.shape == (B, C, H, W)

    if _compiled is None:
        _compiled = _build()
    nc = _compiled

    from concourse.bass_utils import run_bass_kernel_spmd

    in_maps = []
    for k in range(8):
        b, hh = k // 2, k % 2
        sl = slice(96 * hh, 96 * hh + 96)
        in_maps.append(
            {
                "left": np.ascontiguousarray(left[b, :, sl, :]),
                "right": np.ascontiguousarray(right[b, :, sl, :]),
            }
        )

    res = run_bass_kernel_spmd(nc, in_maps, list(range(8)))

    idx, mask = _host_index()
    out = np.zeros((B, D, H, W), dtype=np.float32)
    for k in range(8):
        b, hh = k // 2, k % 2
        # scr is [48 y-pairs, 128 p, 2*NPS*NW]; un-pair to [96, 128*NPS*NW]
        rw = NPS * NW
        scr = (
            res.results[k]["scr"]
            .reshape(HSH // 2, 128, 2, rw)
            .swapaxes(1, 2)
            .reshape(HSH, 128 * rw)
        )
        gathered = scr[:, idx.ravel()].reshape(HSH, D, W)  # [y, d, x]
        gathered *= mask[None, :, :]
        out[b, :, 96 * hh : 96 * hh + 96, :] = gathered.transpose(1, 0, 2)
    return out

